# revision 1
# baseline (speedup 1.0000x reference)
\
"""Trainium2 Bass kernel for nn_Detr3DCrossAttention (DETR3D cross attention).

Sharding: queries are padded 900->1024 and split across 8 NeuronCores (128
queries per core). Each core holds the full multi-level feature maps (as
host-transposed [rows, C] gather tables), computes its queries' camera
projections + bilinear taps on device, gathers 4 taps x 24 (cam,level) pairs
per query with dma_gather, and reduces with static 0/1 selection matrices on
the tensor engine. No collectives needed; host concatenates the 8 outputs.
"""
import os
import numpy as np

import concourse.bass as bass
import concourse.mybir as mybir
import concourse.tile as tile
from concourse import bacc
from concourse.bass import AP
from concourse.masks import make_identity
from concourse import library_config
from concourse.bass_utils import run_bass_kernel_spmd

dt = mybir.dt
Alu = mybir.AluOpType
Act = mybir.ActivationFunctionType
Ax = mybir.AxisListType

# ---- problem constants (hardcoded per spec) ----
PC_RANGE = (-51.2, -51.2, -5.0, 51.2, 51.2, 3.0)
IMG_H, IMG_W = 928, 1600
EPS = 1e-5
LN_EPS = 1e-5
B, Q, D, N, L = 1, 900, 256, 6, 4
LVL_HW = [(116, 200), (58, 100), (29, 50), (15, 25)]
QPAD = 1024
NCORES = 8
QC = QPAD // NCORES  # 128 queries per core

# gather groups: (level, cam_start, cam_end); member order is lev-major cam-minor
GROUPS = [(0, 0, 1), (0, 1, 2), (0, 2, 3), (0, 3, 4), (0, 4, 5), (0, 5, 6),
          (1, 0, 5), (1, 5, 6), (2, 0, 6), (3, 0, 6)]
NMEMB = 24            # total (lev, cam) members
NSLOT = NMEMB * 512   # total gather slots per core (4 taps x 128 q per member)

# reduction matmul dtype: "f32r" (fast), "f32" (exact)
RED_DT = os.environ.get("K_RED_DT", "f32r")



def _host_E():
    E = np.zeros((4, 128, 128), np.float32)
    for j in range(4):
        for p in range(128):
            E[j, p, 32 * j + p // 4] = 1.0
    return E


def _group_of(lev, n):
    for gi, (lv, c0, c1) in enumerate(GROUPS):
        if lv == lev and c0 <= n < c1:
            return gi, n - c0
    raise AssertionError


def _build_program():
    nc = bacc.Bacc("TRN2", target_bir_lowering=False, debug=False)

    # ---------------- DRAM I/O ----------------
    tabs = []
    for gi, (lev, c0, c1) in enumerate(GROUPS):
        H, W = LVL_HW[lev]
        rows = (c1 - c0) * H * W
        tabs.append(nc.dram_tensor(f"tab{gi}", [rows, D], dt.float32,
                                   kind="ExternalInput"))

    def din(name, shape):
        return nc.dram_tensor(name, shape, dt.float32, kind="ExternalInput")

    qT_d = din("qT", [QC, 2, 128])
    qpT_d = din("qpT", [QC, 2, 128])
    rpT_d = din("rpT", [3, QC])          # reference points, q-layout transposed
    rpo_d = din("rpo", [QC, 4, 3])       # reference points, p-ordinal replicated
    l2i_d = din("l2i", [QC, 4, 3, 6])    # lidar2img replicated [p, k, ax, n]
    wattn_d = din("wattn", [128, 2, 24])
    battn_d = din("battn", [QC, 24])
    wout_d = din("wout", [128, 2, D])
    bout_d = din("bout", [QC, D])
    wpe1_d = din("wpe1", [3, D])
    bpe1_d = din("bpe1", [QC, D])
    gpe1_d = din("gpe1", [QC, D])
    bepe1_d = din("bepe1", [QC, D])
    wpe2_d = din("wpe2", [128, 2, D])
    bpe2_d = din("bpe2", [QC, D])
    gpe2_d = din("gpe2", [QC, D])
    bepe2_d = din("bepe2", [QC, D])
    emat_d = nc.dram_tensor("emat", [QC, 4, 128],
                            dt.float32r if RED_DT == "f32r" else dt.float32,
                            kind="ExternalInput")
    rmat_d = din("rmat", [QC, 4, 128])
    # per-partition tap constants [128, 1]: dx, dy, ax=1-dx, bx=2dx-1, ay, by, lox, loy
    tapc_d = din("tapc", [128, 8])
    # per-(partition, level) consts [128, 4, k]: cWs cHs cWp1 cHp1 cWm1 cHm1 hix hiy wmul
    lvlc_d = din("lvlc", [128, 4, 9])
    base_d = din("base", [128, 24])      # member-local gather base per (lev, n)

    out_d = nc.dram_tensor("out", [QC, D], dt.float32, kind="ExternalOutput")
    dbg = os.environ.get("K_DEBUG") == "1"
    if dbg:
        dbg_aw = nc.dram_tensor("dbg_aw", [QC, 24], dt.float32, kind="ExternalOutput")
        dbg_awo = nc.dram_tensor("dbg_awo", [128, 4, 24], dt.float32, kind="ExternalOutput")
        dbg_wfin = nc.dram_tensor("dbg_wfin", [128, 4, 4, 6], dt.float32, kind="ExternalOutput")
        dbg_idx = nc.dram_tensor("dbg_idx", [128, 4, 4, 6], dt.int16, kind="ExternalOutput")
        dbg_wrap = nc.dram_tensor("dbg_wrap", [16, 768], dt.int16, kind="ExternalOutput")
        dbg_fused = nc.dram_tensor("dbg_fused", [QC, D], dt.float32, kind="ExternalOutput")
        dbg_h2r = nc.dram_tensor("dbg_h2r", [QC, D], dt.float32, kind="ExternalOutput")
        dbg_g6 = nc.dram_tensor("dbg_g6", [128, 4, D], dt.float32, kind="ExternalOutput")

    F32 = dt.float32
    red_dt = dt.float32r if RED_DT == "f32r" else dt.float32
    PH = int(os.environ.get("K_PHASE", "6"))

    with tile.TileContext(nc) as tc:
        with tc.tile_pool(name="sb", bufs=1) as sb, \
             tc.tile_pool(name="gpool", bufs=2) as gpool, \
             tc.tile_pool(name="wgpool", bufs=3) as wgpool, \
             tc.tile_pool(name="dramp", bufs=1, space="DRAM") as dramp, \
             tc.tile_pool(name="ps", bufs=1, space="PSUM") as ps, \
             tc.tile_pool(name="pstr", bufs=1, space="PSUM") as pstr:

            V = nc.vector
            S = nc.scalar
            T = nc.tensor
            G = nc.gpsimd

            # ---------------- phase A: load everything ----------------
            def load(name, dram, shape, dtype=F32):
                t = sb.tile(shape, dtype, name=name, tag=name)
                nc.sync.dma_start(t[:], dram[:])
                return t

            qT = load("qT", qT_d, [128, 2, QC])
            qpT = load("qpT", qpT_d, [128, 2, QC])
            rpT = load("rpT", rpT_d, [3, QC])
            rpo = load("rpo", rpo_d, [QC, 4, 3])
            l2i = load("l2i", l2i_d, [QC, 4, 3, 6])
            wattn = load("wattn", wattn_d, [128, 2, 24])
            battn = load("battn", battn_d, [QC, 24])
            wout = load("wout", wout_d, [128, 2, D])
            bout = load("bout", bout_d, [QC, D])
            wpe1 = load("wpe1", wpe1_d, [3, D])
            bpe1 = load("bpe1", bpe1_d, [QC, D])
            gpe1 = load("gpe1", gpe1_d, [QC, D])
            bepe1 = load("bepe1", bepe1_d, [QC, D])
            wpe2 = load("wpe2", wpe2_d, [128, 2, D])
            bpe2 = load("bpe2", bpe2_d, [QC, D])
            gpe2 = load("gpe2", gpe2_d, [QC, D])
            bepe2 = load("bepe2", bepe2_d, [QC, D])
            emat = load("emat", emat_d, [128, 4, 128], dtype=red_dt)
            rmat = load("rmat", rmat_d, [128, 4, 128])
            tapc = load("tapc", tapc_d, [128, 8])
            lvlc = load("lvlc", lvlc_d, [128, 4, 9])
            baset = load("baset", base_d, [128, 24])

            ident = sb.tile([128, 128], F32, name="ident", tag="ident")
            make_identity(nc, ident[:])

            def bc(ap, shape):
                return ap.to_broadcast(shape)

            def ttile(name, shape, dtype=F32):
                return sb.tile(shape, dtype, name=name, tag=name)

            # ---------------- phase B: position-encoder MLP ----------------
            # isig on [3, QC]
            x_cl = ttile("x_cl", [3, QC])
            V.tensor_scalar(out=x_cl[:], in0=rpT[:], scalar1=0.0, scalar2=1.0,
                            op0=Alu.max, op1=Alu.min)
            x1 = ttile("x1", [3, QC])
            V.tensor_scalar(out=x1[:], in0=x_cl[:], scalar1=EPS, scalar2=None,
                            op0=Alu.max)
            x2 = ttile("x2", [3, QC])
            V.tensor_scalar(out=x2[:], in0=x_cl[:], scalar1=-1.0, scalar2=1.0,
                            op0=Alu.mult, op1=Alu.add)
            V.tensor_scalar(out=x2[:], in0=x2[:], scalar1=EPS, scalar2=None,
                            op0=Alu.max)
            rx2 = ttile("rx2", [3, QC])
            V.reciprocal(rx2[:], x2[:])
            ratio = ttile("ratio", [3, QC])
            V.tensor_tensor(out=ratio[:], in0=x1[:], in1=rx2[:], op=Alu.mult)
            isig = ttile("isig", [3, QC])
            S.activation(isig[:], ratio[:], Act.Ln)

            def layernorm(x_sb, g_t, be_t, name):
                mu = ttile(f"{name}_mu", [QC, 1])
                V.tensor_reduce(out=mu[:], in_=x_sb[:], axis=Ax.X, op=Alu.add)
                V.tensor_scalar(out=mu[:], in0=mu[:], scalar1=1.0 / D,
                                scalar2=None, op0=Alu.mult)
                xc = ttile(f"{name}_xc", [QC, D])
                V.tensor_scalar(out=xc[:], in0=x_sb[:], scalar1=mu[:, 0:1],
                                scalar2=None, op0=Alu.subtract)
                sq = ttile(f"{name}_sq", [QC, D])
                var = ttile(f"{name}_var", [QC, 1])
                V.tensor_tensor(out=sq[:], in0=xc[:], in1=xc[:], op=Alu.mult)
                V.tensor_reduce(out=var[:], in_=sq[:], axis=Ax.X, op=Alu.add)
                V.tensor_scalar(out=var[:], in0=var[:], scalar1=1.0 / D,
                                scalar2=LN_EPS, op0=Alu.mult, op1=Alu.add)
                sd = ttile(f"{name}_sd", [QC, 1])
                S.activation(sd[:], var[:], Act.Sqrt)
                rs = ttile(f"{name}_rs", [QC, 1])
                V.reciprocal(rs[:], sd[:])
                V.tensor_scalar(out=xc[:], in0=xc[:], scalar1=rs[:, 0:1],
                                scalar2=None, op0=Alu.mult)
                V.tensor_tensor(out=xc[:], in0=xc[:], in1=g_t[:], op=Alu.mult)
                V.tensor_tensor(out=xc[:], in0=xc[:], in1=be_t[:], op=Alu.add)
                return xc

            h1_ps = ps.tile([QC, D], F32, name="h1_ps", tag="h1_ps")
            T.matmul(out=h1_ps[:], lhsT=isig[:], rhs=wpe1[:], start=True, stop=True)
            h1 = ttile("h1", [QC, D])
            V.tensor_tensor(out=h1[:], in0=h1_ps[:], in1=bpe1[:], op=Alu.add)
            h1n = layernorm(h1, gpe1, bepe1, "ln1")
            h1r = ttile("h1r", [QC, D])
            S.activation(h1r[:], h1n[:], Act.Relu)

            h1T = ttile("h1T", [128, 2, QC])
            for c in range(2):
                trp = pstr.tile([128, 128], F32, name="trp", tag="trp")
                T.transpose(out=trp[:], in_=h1r[:, c * 128:(c + 1) * 128],
                            identity=ident[:])
                V.tensor_copy(h1T[:, c, :], trp[:])

            h2_ps = ps.tile([QC, D], F32, name="h2_ps", tag="h2_ps")
            for c in range(2):
                T.matmul(out=h2_ps[:], lhsT=h1T[:, c, :], rhs=wpe2[:, c, :],
                         start=(c == 0), stop=(c == 1))
            h2 = ttile("h2", [QC, D])
            V.tensor_tensor(out=h2[:], in0=h2_ps[:], in1=bpe2[:], op=Alu.add)
            h2n = layernorm(h2, gpe2, bepe2, "ln2")
            h2r = ttile("h2r", [QC, D])
            S.activation(h2r[:], h2n[:], Act.Relu)

            # ---------------- phase C: attention weights ----------------
            qsT = ttile("qsT", [128, 2, QC])
            V.tensor_tensor(out=qsT[:], in0=qT[:], in1=qpT[:], op=Alu.add)
            aw_ps = ps.tile([QC, 24], F32, name="aw_ps", tag="aw_ps")
            for c in range(2):
                T.matmul(out=aw_ps[:], lhsT=qsT[:, c, :], rhs=wattn[:, c, :],
                         start=(c == 0), stop=(c == 1))
            awl = ttile("awl", [QC, 24])
            V.tensor_tensor(out=awl[:], in0=aw_ps[:], in1=battn[:], op=Alu.add)
            aw = ttile("aw", [QC, 24])
            S.activation(aw[:], awl[:], Act.Sigmoid)

            awo_ps = ps.tile([128, 4, 24], F32, name="awo_ps", tag="awo_ps")
            for j in range(4):
                T.matmul(out=awo_ps[:, j, :], lhsT=rmat[:, j, :], rhs=aw[:],
                         start=True, stop=True)
            awo = ttile("awo", [128, 4, 24])
            V.tensor_copy(awo[:], awo_ps[:])

            # ---------------- phase D: coordinate pipeline (p-ordinal) -------
            pr = PC_RANGE
            rw = ttile("rw", [128, 4, 3])
            for k in range(3):
                V.tensor_scalar(out=rw[:, :, k:k + 1], in0=rpo[:, :, k:k + 1],
                                scalar1=float(pr[3 + k] - pr[k]),
                                scalar2=float(pr[k]), op0=Alu.mult, op1=Alu.add)

            cam3 = ttile("cam3", [128, 4, 3, 6])
            tmp3 = ttile("tmp3", [128, 4, 3, 6])
            sh = [128, 4, 3, 6]
            V.tensor_tensor(out=cam3[:], in0=bc(rw[:, :, 0:1].unsqueeze(3), sh),
                            in1=bc(l2i[:, 0:1, :, :], sh), op=Alu.mult)
            V.tensor_tensor(out=tmp3[:], in0=bc(rw[:, :, 1:2].unsqueeze(3), sh),
                            in1=bc(l2i[:, 1:2, :, :], sh), op=Alu.mult)
            V.tensor_tensor(out=cam3[:], in0=cam3[:], in1=tmp3[:], op=Alu.add)
            V.tensor_tensor(out=tmp3[:], in0=bc(rw[:, :, 2:3].unsqueeze(3), sh),
                            in1=bc(l2i[:, 2:3, :, :], sh), op=Alu.mult)
            V.tensor_tensor(out=cam3[:], in0=cam3[:], in1=tmp3[:], op=Alu.add)
            V.tensor_tensor(out=cam3[:], in0=cam3[:], in1=bc(l2i[:, 3:4, :, :], sh),
                            op=Alu.add)

            zc = ttile("zc", [128, 4, 6])
            V.tensor_scalar(out=zc[:], in0=cam3[:, :, 2, :], scalar1=EPS,
                            scalar2=None, op0=Alu.max)
            rz = ttile("rz", [128, 4, 6])
            V.reciprocal(rz[:], zc[:])
            xr = ttile("xr", [128, 4, 6])
            V.tensor_tensor(out=xr[:], in0=cam3[:, :, 0, :], in1=rz[:], op=Alu.mult)
            yr = ttile("yr", [128, 4, 6])
            V.tensor_tensor(out=yr[:], in0=cam3[:, :, 1, :], in1=rz[:], op=Alu.mult)

            # mask = (z > eps) & (0 < xr < W) & (0 < yr < H)
            msk = ttile("msk", [128, 4, 6])
            mt = ttile("mt", [128, 4, 6])
            V.tensor_scalar(out=msk[:], in0=cam3[:, :, 2, :], scalar1=EPS,
                            scalar2=None, op0=Alu.is_gt)
            V.tensor_scalar(out=mt[:], in0=xr[:], scalar1=0.0, scalar2=None,
                            op0=Alu.is_gt)
            V.tensor_tensor(out=msk[:], in0=msk[:], in1=mt[:], op=Alu.mult)
            V.tensor_scalar(out=mt[:], in0=xr[:], scalar1=float(IMG_W),
                            scalar2=None, op0=Alu.is_lt)
            V.tensor_tensor(out=msk[:], in0=msk[:], in1=mt[:], op=Alu.mult)
            V.tensor_scalar(out=mt[:], in0=yr[:], scalar1=0.0, scalar2=None,
                            op0=Alu.is_gt)
            V.tensor_tensor(out=msk[:], in0=msk[:], in1=mt[:], op=Alu.mult)
            V.tensor_scalar(out=mt[:], in0=yr[:], scalar1=float(IMG_H),
                            scalar2=None, op0=Alu.is_lt)
            V.tensor_tensor(out=msk[:], in0=msk[:], in1=mt[:], op=Alu.mult)

            # maw[p, j, lev, n] = mask * aw (aw cols are n*4+l)
            sh4 = [128, 4, 4, 6]
            maw = ttile("maw", sh4)
            aw_r = awo[:].rearrange("p j (n l) -> p j l n", n=6, l=4)
            V.tensor_tensor(out=maw[:], in0=bc(msk[:].unsqueeze(2), sh4),
                            in1=aw_r, op=Alu.mult)

            # level-batched tap computation; lvlc cols:
            # 0 cWs, 1 cHs, 2 cWp1, 3 cHp1, 4 cWm1, 5 cHm1, 6 hix, 7 hiy, 8 wmul
            def lc(k):
                return bc(lvlc[:, :, k:k + 1].transpose([0, 2, 1]).unsqueeze(3), sh4)

            def flat(t):
                return t[:].rearrange("p a b c -> p (a b c)")

            def tapc_bf(k):
                return tapc[:, k:k + 1].to_broadcast([128, 96])

            def tapc_b(k):
                return bc(tapc[:, k:k + 1].unsqueeze(2).unsqueeze(3), sh4)

            xr_b = bc(xr[:].unsqueeze(2), sh4)
            yr_b = bc(yr[:].unsqueeze(2), sh4)

            pxc = ttile("pxc", sh4)
            pyc = ttile("pyc", sh4)
            V.tensor_tensor(out=pxc[:], in0=xr_b, in1=lc(0), op=Alu.mult)
            V.tensor_scalar(out=flat(pxc), in0=flat(pxc), scalar1=-0.5,
                            scalar2=-2.0, op0=Alu.add, op1=Alu.max)
            V.tensor_tensor(out=pxc[:], in0=pxc[:], in1=lc(2), op=Alu.min)
            V.tensor_tensor(out=pyc[:], in0=yr_b, in1=lc(1), op=Alu.mult)
            V.tensor_scalar(out=flat(pyc), in0=flat(pyc), scalar1=-0.5,
                            scalar2=-2.0, op0=Alu.add, op1=Alu.max)
            V.tensor_tensor(out=pyc[:], in0=pyc[:], in1=lc(3), op=Alu.min)

            # floor via int cast (robust to either rounding mode)
            def floor_frac(pc, name):
                ii = sb.tile(sh4, dt.int32, name=f"{name}_i", tag=f"{name}_i")
                V.tensor_copy(flat(ii), flat(pc))
                ff = ttile(f"{name}_f", sh4)
                V.tensor_copy(flat(ff), flat(ii))
                dg = ttile(f"{name}_d", sh4)
                V.tensor_tensor(out=flat(dg), in0=flat(ff), in1=flat(pc), op=Alu.is_gt)
                f0 = ttile(f"{name}_0", sh4)
                V.tensor_tensor(out=flat(f0), in0=flat(ff), in1=flat(dg), op=Alu.subtract)
                wf = ttile(f"{name}_w", sh4)
                V.tensor_tensor(out=flat(wf), in0=flat(pc), in1=flat(f0), op=Alu.subtract)
                return f0, wf

            x0, wx = floor_frac(pxc, "fx")
            y0, wy = floor_frac(pyc, "fy")

            xt = ttile("xt", sh4)
            V.tensor_tensor(out=flat(xt), in0=flat(x0), in1=tapc_bf(0), op=Alu.add)
            yt = ttile("yt", sh4)
            V.tensor_tensor(out=flat(yt), in0=flat(y0), in1=tapc_bf(1), op=Alu.add)

            wxt = ttile("wxt", sh4)
            V.scalar_tensor_tensor(out=flat(wxt), in0=flat(wx), scalar=tapc[:, 3:4],
                                   in1=tapc_bf(2), op0=Alu.mult, op1=Alu.add)
            wyt = ttile("wyt", sh4)
            V.scalar_tensor_tensor(out=flat(wyt), in0=flat(wy), scalar=tapc[:, 5:6],
                                   in1=tapc_bf(4), op0=Alu.mult, op1=Alu.add)

            cxl = ttile("cxl", sh4)
            V.tensor_scalar(out=flat(cxl), in0=flat(pxc), scalar1=tapc[:, 6:7],
                            scalar2=None, op0=Alu.is_ge)
            cxh = ttile("cxh", sh4)
            V.tensor_tensor(out=cxh[:], in0=pxc[:], in1=lc(6), op=Alu.is_lt)
            cyl = ttile("cyl", sh4)
            V.tensor_scalar(out=flat(cyl), in0=flat(pyc), scalar1=tapc[:, 7:8],
                            scalar2=None, op0=Alu.is_ge)
            cyh = ttile("cyh", sh4)
            V.tensor_tensor(out=cyh[:], in0=pyc[:], in1=lc(7), op=Alu.is_lt)

            wfin = ttile("wfin", sh4)
            V.tensor_tensor(out=flat(cxl), in0=flat(cxl), in1=flat(cxh), op=Alu.mult)
            V.tensor_tensor(out=flat(cyl), in0=flat(cyl), in1=flat(cyh), op=Alu.mult)
            V.tensor_tensor(out=flat(cxl), in0=flat(cxl), in1=flat(cyl), op=Alu.mult)
            V.tensor_tensor(out=flat(wfin), in0=flat(wxt), in1=flat(wyt), op=Alu.mult)
            V.tensor_tensor(out=flat(wfin), in0=flat(wfin), in1=flat(cxl), op=Alu.mult)
            V.tensor_tensor(out=flat(wfin), in0=flat(wfin), in1=flat(maw), op=Alu.mult)

            xcl = ttile("xcl", sh4)
            V.tensor_scalar(out=flat(xcl), in0=flat(xt), scalar1=0.0, scalar2=None,
                            op0=Alu.max)
            V.tensor_tensor(out=xcl[:], in0=xcl[:], in1=lc(4), op=Alu.min)
            ycl = ttile("ycl", sh4)
            V.tensor_scalar(out=flat(ycl), in0=flat(yt), scalar1=0.0, scalar2=None,
                            op0=Alu.max)
            V.tensor_tensor(out=ycl[:], in0=ycl[:], in1=lc(5), op=Alu.min)

            idxf = ttile("idxf", sh4)
            V.tensor_tensor(out=idxf[:], in0=ycl[:], in1=lc(8), op=Alu.mult)
            V.tensor_tensor(out=flat(idxf), in0=flat(idxf), in1=flat(xcl), op=Alu.add)
            base_b = baset[:].rearrange("p (l n) -> p l n", l=4, n=6).unsqueeze(1)
            V.tensor_tensor(out=idxf[:], in0=idxf[:], in1=bc(base_b, sh4),
                            op=Alu.add)
            idx16 = sb.tile([128, 4, 4, 6], dt.int16, name="idx16", tag="idx16")
            V.tensor_copy(flat(idx16), flat(idxf))

            # ---------------- phase E: wrap indices for dma_gather -----------
            bounce = dramp.tile([1, NSLOT], dt.int16, name="bounce", tag="bounce")
            bh = bounce.tensor
            # dst flat slot = 512*(lev*6+n) + 128*j + p; one DMA per tap j so
            # both sides stay <=3 AP dims
            for j in range(4):
                nc.sync.dma_start(
                    AP(bh, 128 * j, [[1, 128], [512, 24]]),
                    idx16[:, j, :, :])
            wrap = sb.tile([128, NSLOT // 16], dt.int16, name="wrap", tag="wrap")
            for k in range(8):
                nc.sync.dma_start(
                    wrap[16 * k:16 * (k + 1), :],
                    AP(bh, 0, [[1, 16], [16, NSLOT // 16]]))

            # ---------------- phase F: gather + weight + reduce --------------
            G.load_library(library_config.mlp)
            fused_ps = ps.tile([QC, D], F32, name="fused_ps", tag="fused_ps")
            n_mm = 96
            mm_i = 0
            for gi, (lev, c0, c1) in enumerate(GROUPS):
                nm = c1 - c0
                nidx = nm * 512
                m0 = lev * 6 + c0
                gout = gpool.tile([128, 4 * nm, D], F32, name=f"g{gi}", tag="gout",
                                  padded_shape=[128, 24, D])
                if os.environ.get("K_NOGATHER") == "1":
                    V.memset(gout[:], 0.25)
                else:
                    G.dma_gather(
                        out_ap=gout[:],
                        in_ap=tabs[gi][:],
                        idxs_ap=wrap[:, m0 * 32:(m0 + nm) * 32],
                        num_idxs=nidx,
                        num_idxs_reg=nidx,
                        elem_size=D,
                        single_packet=False,
                    )
                if dbg and gi == 0:
                    nc.sync.dma_start(dbg_g6[:], gout[:, 0:4, :])
                for mloc in range(nm):
                    n_cam = c0 + mloc
                    wg = wgpool.tile([128, 4, D], red_dt, name=f"wg{gi}_{mloc}",
                                     tag="wg")
                    for j in range(4):
                        V.tensor_scalar(out=wg[:, j, :],
                                        in0=gout[:, 4 * mloc + j, :],
                                        scalar1=wfin[:, j, lev, n_cam:n_cam + 1],
                                        scalar2=None, op0=Alu.mult)
                    for j in range(4):
                        T.matmul(out=fused_ps[:],
                                 lhsT=emat[:, j, :], rhs=wg[:, j, :],
                                 start=(mm_i == 0), stop=(mm_i == n_mm - 1))
                        mm_i += 1

            # ---------------- phase G: output projection ---------------------
            fused = ttile("fused", [QC, D])
            V.tensor_copy(fused[:], fused_ps[:])
            fusedT = ttile("fusedT", [128, 2, QC])
            for c in range(2):
                trp2 = pstr.tile([128, 128], F32, name="trp2", tag="trp")
                T.transpose(out=trp2[:], in_=fused[:, c * 128:(c + 1) * 128],
                            identity=ident[:])
                V.tensor_copy(fusedT[:, c, :], trp2[:])
            out_ps = ps.tile([QC, D], F32, name="out_ps", tag="out_ps")
            for c in range(2):
                T.matmul(out=out_ps[:], lhsT=fusedT[:, c, :], rhs=wout[:, c, :],
                         start=(c == 0), stop=(c == 1))
            if dbg:
                nc.sync.dma_start(dbg_aw[:], aw[:])
                nc.sync.dma_start(dbg_awo[:], awo[:])
                nc.sync.dma_start(dbg_wfin[:], wfin[:])
                nc.sync.dma_start(dbg_idx[:], idx16[:])
                nc.sync.dma_start(dbg_wrap[:], wrap[0:16, :])
                nc.sync.dma_start(dbg_fused[:], fused[:])
                nc.sync.dma_start(dbg_h2r[:], h2r[:])
            o1 = ttile("o1", [QC, D])
            V.tensor_tensor(out=o1[:], in0=out_ps[:], in1=bout[:], op=Alu.add)
            V.tensor_tensor(out=o1[:], in0=o1[:], in1=h2r[:], op=Alu.add)
            nc.sync.dma_start(out_d[:], o1[:])

    nc.compile()
    return nc


_NC_CACHE = None


def _get_program():
    global _NC_CACHE
    if _NC_CACHE is None:
        _NC_CACHE = _build_program()
    return _NC_CACHE


def _host_prep(inputs):
    """Build the shared (core-independent) and per-core input maps."""
    f32 = np.float32
    query = np.asarray(inputs["query"], f32)[0]
    query_pos = np.asarray(inputs["query_pos"], f32)[0]
    rp = np.asarray(inputs["reference_points"], f32)[0]
    l2i = np.asarray(inputs["lidar2img"], f32)[0]
    feats = [np.asarray(inputs[f"feat{i}"], f32)[0] for i in range(4)]

    def padq(x, fill):
        out = np.full((QPAD,) + x.shape[1:], fill, f32)
        out[:Q] = x
        return out

    query_p = padq(query, 0.0)
    qpos_p = padq(query_pos, 0.0)
    rp_p = padq(rp, 0.5)

    shared = {}
    for gi, (lev, c0, c1) in enumerate(GROUPS):
        t = feats[lev][c0:c1]
        ncam, C, H, W = t.shape
        shared[f"tab{gi}"] = np.ascontiguousarray(
            t.transpose(0, 2, 3, 1).reshape(ncam * H * W, C))

    # static matrices / constants
    E = _host_E()
    shared["emat"] = np.ascontiguousarray(E.transpose(1, 0, 2))  # [p, j, q]
    shared["rmat"] = np.ascontiguousarray(E.transpose(2, 0, 1))  # [q, j, p]

    pvec = np.arange(128)
    dx = (pvec % 4 % 2).astype(f32)
    dy = (pvec % 4 // 2).astype(f32)
    tapc = np.stack([dx, dy, 1 - dx, 2 * dx - 1, 1 - dy, 2 * dy - 1,
                     np.where(dx > 0.5, -1.0, 0.0),
                     np.where(dy > 0.5, -1.0, 0.0)], axis=1).astype(f32)
    shared["tapc"] = tapc

    lvlc = np.zeros((128, 4, 9), f32)
    for lv, (H, W) in enumerate(LVL_HW):
        lvlc[:, lv, 0] = W / IMG_W
        lvlc[:, lv, 1] = H / IMG_H
        lvlc[:, lv, 2] = W + 1.0
        lvlc[:, lv, 3] = H + 1.0
        lvlc[:, lv, 4] = W - 1.0
        lvlc[:, lv, 5] = H - 1.0
        lvlc[:, lv, 6] = np.where(dx > 0.5, W - 1.0, float(W))
        lvlc[:, lv, 7] = np.where(dy > 0.5, H - 1.0, float(H))
        lvlc[:, lv, 8] = float(W)
    shared["lvlc"] = lvlc

    base = np.zeros((128, 24), f32)
    for lv in range(4):
        H, W = LVL_HW[lv]
        for n in range(6):
            gi, mloc = _group_of(lv, n)
            base[:, lv * 6 + n] = mloc * H * W
    shared["base"] = base

    shared["wattn"] = np.ascontiguousarray(
        np.asarray(inputs["W_attn"], f32).reshape(2, 128, 24).transpose(1, 0, 2))
    shared["battn"] = np.broadcast_to(
        np.asarray(inputs["b_attn"], f32), (QC, 24)).copy()
    shared["wout"] = np.ascontiguousarray(
        np.asarray(inputs["W_out"], f32).reshape(2, 128, D).transpose(1, 0, 2))
    shared["bout"] = np.broadcast_to(
        np.asarray(inputs["b_out"], f32), (QC, D)).copy()
    shared["wpe1"] = np.asarray(inputs["W_pe1"], f32)
    shared["wpe2"] = np.ascontiguousarray(
        np.asarray(inputs["W_pe2"], f32).reshape(2, 128, D).transpose(1, 0, 2))
    for nm, key in [("bpe1", "b_pe1"), ("gpe1", "g_pe1"), ("bepe1", "be_pe1"),
                    ("bpe2", "b_pe2"), ("gpe2", "g_pe2"), ("bepe2", "be_pe2")]:
        shared[nm] = np.broadcast_to(
            np.asarray(inputs[key], f32), (QC, D)).copy()

    # l2i replicated: [p, k, ax, n] = l2i[n, ax, k]
    l2i_r = np.broadcast_to(
        l2i.transpose(2, 1, 0)[:, :3, :][None], (QC, 4, 3, 6)).copy()
    shared["l2i"] = np.ascontiguousarray(l2i_r)

    in_maps = []
    ordmap = (32 * (pvec[:, None] * 0 + np.arange(4)[None, :]) +
              pvec[:, None] // 4)  # [128, 4] local ordinal -> local q
    for cid in range(NCORES):
        q0 = cid * QC
        m = dict(shared)
        m["qT"] = np.ascontiguousarray(
            query_p[q0:q0 + QC].T.reshape(2, 128, QC).transpose(1, 0, 2))
        m["qpT"] = np.ascontiguousarray(
            qpos_p[q0:q0 + QC].T.reshape(2, 128, QC).transpose(1, 0, 2))
        m["rpT"] = np.ascontiguousarray(rp_p[q0:q0 + QC].T)
        m["rpo"] = np.ascontiguousarray(rp_p[q0 + ordmap])
        in_maps.append(m)
    return in_maps


def kernel(**inputs):
    nc = _get_program()
    in_maps = _host_prep(inputs)
    res = run_bass_kernel_spmd(nc, in_maps, core_ids=list(range(NCORES)))
    outs = [res.results[cid]["out"] for cid in range(NCORES)]
    full = np.concatenate(outs, axis=0)[:Q]
    return full[None].astype(np.float32)


def kernel_traced(**inputs):
    """test.py helper: also returns exec_time_ns from the NTFF profile."""
    nc = _get_program()
    in_maps = _host_prep(inputs)
    res = run_bass_kernel_spmd(nc, in_maps, core_ids=list(range(NCORES)),
                               trace=True)
    outs = [res.results[cid]["out"] for cid in range(NCORES)]
    full = np.concatenate(outs, axis=0)[:Q]
    return full[None].astype(np.float32), res



# revision 11
# speedup vs baseline: 5.6413x; 5.6413x over previous
\
"""Trainium2 Bass kernel for nn_Detr3DCrossAttention (DETR3D cross attention).

Sharding: queries are padded 900->1024 and split across 8 NeuronCores (128
queries per core). Each core holds the full multi-level feature maps (as
host-transposed [rows, C] gather tables), computes its queries' camera
projections + bilinear taps on device, gathers 4 taps x 24 (cam,level) pairs
per query with dma_gather, and reduces with static 0/1 selection matrices on
the tensor engine. No collectives needed; host concatenates the 8 outputs.
"""
import os
import numpy as np

import concourse.bass as bass
import concourse.mybir as mybir
import concourse.tile as tile
from concourse import bacc
from concourse.bass import AP
from concourse.masks import make_identity
from concourse import library_config
from concourse.bass_utils import run_bass_kernel_spmd

dt = mybir.dt
Alu = mybir.AluOpType
Act = mybir.ActivationFunctionType
Ax = mybir.AxisListType

# ---- problem constants (hardcoded per spec) ----
PC_RANGE = (-51.2, -51.2, -5.0, 51.2, 51.2, 3.0)
IMG_H, IMG_W = 928, 1600
EPS = 1e-5
LN_EPS = 1e-5
B, Q, D, N, L = 1, 900, 256, 6, 4
LVL_HW = [(116, 200), (58, 100), (29, 50), (15, 25)]
QPAD = 1024
NCORES = 8
QC = QPAD // NCORES  # 128 queries per core

# gather groups: (level, cam_start, cam_end); member order is lev-major cam-minor
GROUPS = [(0, 0, 1), (0, 1, 2), (0, 2, 3), (0, 3, 4), (0, 4, 5), (0, 5, 6),
          (1, 0, 5), (1, 5, 6), (2, 0, 6), (3, 0, 6)]
NMEMB = 24            # total (lev, cam) members
NSLOT = NMEMB * 512   # total gather slots per core (4 taps x 128 q per member)

# reduction matmul dtype: "f32r" (fast), "f32" (exact)
RED_DT = os.environ.get("K_RED_DT", "f32r")



def _host_E():
    E = np.zeros((4, 128, 128), np.float32)
    for j in range(4):
        for p in range(128):
            E[j, p, 32 * j + p // 4] = 1.0
    return E


def _group_of(lev, n):
    for gi, (lv, c0, c1) in enumerate(GROUPS):
        if lv == lev and c0 <= n < c1:
            return gi, n - c0
    raise AssertionError


def _build_program():
    nc = bacc.Bacc("TRN2", target_bir_lowering=False, debug=False)

    # ---------------- DRAM I/O ----------------
    tabs = []
    for gi, (lev, c0, c1) in enumerate(GROUPS):
        H, W = LVL_HW[lev]
        rows = (c1 - c0) * H * W
        tabs.append(nc.dram_tensor(f"tab{gi}", [rows, D], dt.bfloat16,
                                   kind="ExternalInput"))

    def din(name, shape):
        return nc.dram_tensor(name, shape, dt.float32, kind="ExternalInput")

    qT_d = din("qT", [QC, 2, 128])
    qpT_d = din("qpT", [QC, 2, 128])
    rpT_d = din("rpT", [3, QC])          # reference points, q-layout transposed
    rpo_d = din("rpo", [QC, 4, 3])       # reference points, p-ordinal replicated
    l2i_d = din("l2i", [QC, 4, 3, 6])    # lidar2img replicated [p, k, ax, n]
    wattn_d = din("wattn", [128, 2, 24])
    battn_d = din("battn", [QC, 24])
    wout_d = din("wout", [128, 2, D])
    bout_d = din("bout", [QC, D])
    wpe1_d = din("wpe1", [3, D])
    bpe1_d = din("bpe1", [QC, D])
    gpe1_d = din("gpe1", [QC, D])
    bepe1_d = din("bepe1", [QC, D])
    wpe2_d = din("wpe2", [128, 2, D])
    bpe2_d = din("bpe2", [QC, D])
    gpe2_d = din("gpe2", [QC, D])
    bepe2_d = din("bepe2", [QC, D])
    emat_d = nc.dram_tensor("emat", [QC, 4, 128],
                            dt.float32r if RED_DT == "f32r" else dt.float32,
                            kind="ExternalInput")
    rmat_d = din("rmat", [QC, 4, 128])
    # per-partition tap constants [128, 1]: dx, dy, ax=1-dx, bx=2dx-1, ay, by, lox, loy
    tapc_d = din("tapc", [128, 8])
    # per-(partition, level) consts [128, 4, k]: cWs cHs cWp1 cHp1 cWm1 cHm1 hix hiy wmul
    lvlc_d = din("lvlc", [128, 4, 9])
    base_d = din("base", [128, 24])      # member-local gather base per (lev, n)
    smat_d = din("smat", [128, 8, 16])   # wrap partition-shuffle selectors

    out_d = nc.dram_tensor("out", [QC, D], dt.float32, kind="ExternalOutput")
    dbg = os.environ.get("K_DEBUG") == "1"
    if dbg:
        dbg_aw = nc.dram_tensor("dbg_aw", [QC, 24], dt.float32, kind="ExternalOutput")
        dbg_awo = nc.dram_tensor("dbg_awo", [128, 4, 24], dt.float32, kind="ExternalOutput")
        dbg_wfin = nc.dram_tensor("dbg_wfin", [128, 4, 4, 6], dt.float32, kind="ExternalOutput")
        dbg_idx = nc.dram_tensor("dbg_idx", [128, 4, 4, 6], dt.float32, kind="ExternalOutput")
        dbg_wrap = nc.dram_tensor("dbg_wrap", [16, 768], dt.int16, kind="ExternalOutput")
        dbg_fused = nc.dram_tensor("dbg_fused", [QC, D], dt.float32, kind="ExternalOutput")
        dbg_h2r = nc.dram_tensor("dbg_h2r", [QC, D], dt.float32, kind="ExternalOutput")
        dbg_g6 = nc.dram_tensor("dbg_g6", [128, 4, D], dt.bfloat16, kind="ExternalOutput")

    F32 = dt.float32
    red_dt = dt.float32r if RED_DT == "f32r" else dt.float32
    PH = int(os.environ.get("K_PHASE", "6"))

    with tile.TileContext(nc) as tc:
        with tc.tile_pool(name="sb", bufs=1) as sb, \
             tc.tile_pool(name="gpool", bufs=2) as gpool, \
             tc.tile_pool(name="wgpool", bufs=3) as wgpool, \
             tc.tile_pool(name="dramp", bufs=1, space="DRAM") as dramp, \
             tc.tile_pool(name="ps", bufs=1, space="PSUM") as ps, \
             tc.tile_pool(name="pstr", bufs=1, space="PSUM") as pstr:

            V = nc.vector
            S = nc.scalar
            T = nc.tensor
            G = nc.gpsimd

            # ---------------- phase A: load everything ----------------
            def load(name, dram, shape, dtype=F32):
                t = sb.tile(shape, dtype, name=name, tag=name)
                nc.sync.dma_start(t[:], dram[:])
                return t

            qT = load("qT", qT_d, [128, 2, QC])
            qpT = load("qpT", qpT_d, [128, 2, QC])
            rpT = load("rpT", rpT_d, [3, QC])
            rpo = load("rpo", rpo_d, [QC, 4, 3])
            l2i = load("l2i", l2i_d, [QC, 4, 3, 6])
            wattn = load("wattn", wattn_d, [128, 2, 24])
            battn = load("battn", battn_d, [QC, 24])
            wout = load("wout", wout_d, [128, 2, D])
            bout = load("bout", bout_d, [QC, D])
            wpe1 = load("wpe1", wpe1_d, [3, D])
            bpe1 = load("bpe1", bpe1_d, [QC, D])
            gpe1 = load("gpe1", gpe1_d, [QC, D])
            bepe1 = load("bepe1", bepe1_d, [QC, D])
            wpe2 = load("wpe2", wpe2_d, [128, 2, D])
            bpe2 = load("bpe2", bpe2_d, [QC, D])
            gpe2 = load("gpe2", gpe2_d, [QC, D])
            bepe2 = load("bepe2", bepe2_d, [QC, D])
            emat = load("emat", emat_d, [128, 4, 128], dtype=red_dt)
            rmat = load("rmat", rmat_d, [128, 4, 128])
            tapc = load("tapc", tapc_d, [128, 8])
            lvlc = load("lvlc", lvlc_d, [128, 4, 9])
            baset = load("baset", base_d, [128, 24])
            smat = load("smat", smat_d, [128, 8, 16])

            ident = sb.tile([128, 128], F32, name="ident", tag="ident")
            make_identity(nc, ident[:])

            def bc(ap, shape):
                return ap.to_broadcast(shape)

            def ttile(name, shape, dtype=F32):
                return sb.tile(shape, dtype, name=name, tag=name)

            # ---------------- phase B: position-encoder MLP ----------------
            # isig on [3, QC]
            x_cl = ttile("x_cl", [3, QC])
            V.tensor_scalar(out=x_cl[:], in0=rpT[:], scalar1=0.0, scalar2=1.0,
                            op0=Alu.max, op1=Alu.min)
            x1 = ttile("x1", [3, QC])
            V.tensor_scalar(out=x1[:], in0=x_cl[:], scalar1=EPS, scalar2=None,
                            op0=Alu.max)
            x2 = ttile("x2", [3, QC])
            V.tensor_scalar(out=x2[:], in0=x_cl[:], scalar1=-1.0, scalar2=1.0,
                            op0=Alu.mult, op1=Alu.add)
            V.tensor_scalar(out=x2[:], in0=x2[:], scalar1=EPS, scalar2=None,
                            op0=Alu.max)
            rx2 = ttile("rx2", [3, QC])
            V.reciprocal(rx2[:], x2[:])
            ratio = ttile("ratio", [3, QC])
            V.tensor_tensor(out=ratio[:], in0=x1[:], in1=rx2[:], op=Alu.mult)
            isig = ttile("isig", [3, QC])
            S.activation(isig[:], ratio[:], Act.Ln)

            def layernorm(x_sb, g_t, be_t, name):
                mu = ttile(f"{name}_mu", [QC, 1])
                V.tensor_reduce(out=mu[:], in_=x_sb[:], axis=Ax.X, op=Alu.add)
                V.tensor_scalar(out=mu[:], in0=mu[:], scalar1=1.0 / D,
                                scalar2=None, op0=Alu.mult)
                xc = ttile(f"{name}_xc", [QC, D])
                V.tensor_scalar(out=xc[:], in0=x_sb[:], scalar1=mu[:, 0:1],
                                scalar2=None, op0=Alu.subtract)
                sq = ttile(f"{name}_sq", [QC, D])
                var = ttile(f"{name}_var", [QC, 1])
                V.tensor_tensor(out=sq[:], in0=xc[:], in1=xc[:], op=Alu.mult)
                V.tensor_reduce(out=var[:], in_=sq[:], axis=Ax.X, op=Alu.add)
                V.tensor_scalar(out=var[:], in0=var[:], scalar1=1.0 / D,
                                scalar2=LN_EPS, op0=Alu.mult, op1=Alu.add)
                sd = ttile(f"{name}_sd", [QC, 1])
                S.activation(sd[:], var[:], Act.Sqrt)
                rs = ttile(f"{name}_rs", [QC, 1])
                V.reciprocal(rs[:], sd[:])
                V.tensor_scalar(out=xc[:], in0=xc[:], scalar1=rs[:, 0:1],
                                scalar2=None, op0=Alu.mult)
                V.tensor_tensor(out=xc[:], in0=xc[:], in1=g_t[:], op=Alu.mult)
                V.tensor_tensor(out=xc[:], in0=xc[:], in1=be_t[:], op=Alu.add)
                return xc

            h1_ps = ps.tile([QC, D], F32, name="h1_ps", tag="h1_ps")
            T.matmul(out=h1_ps[:], lhsT=isig[:], rhs=wpe1[:], start=True, stop=True)
            h1 = ttile("h1", [QC, D])
            V.tensor_tensor(out=h1[:], in0=h1_ps[:], in1=bpe1[:], op=Alu.add)
            h1n = layernorm(h1, gpe1, bepe1, "ln1")
            h1r = ttile("h1r", [QC, D])
            S.activation(h1r[:], h1n[:], Act.Relu)

            h1T = ttile("h1T", [128, 2, QC])
            for c in range(2):
                trp = pstr.tile([128, 128], F32, name="trp", tag="trp")
                T.transpose(out=trp[:], in_=h1r[:, c * 128:(c + 1) * 128],
                            identity=ident[:])
                V.tensor_copy(h1T[:, c, :], trp[:])

            h2_ps = ps.tile([QC, D], F32, name="h2_ps", tag="h2_ps")
            for c in range(2):
                T.matmul(out=h2_ps[:], lhsT=h1T[:, c, :], rhs=wpe2[:, c, :],
                         start=(c == 0), stop=(c == 1))
            h2 = ttile("h2", [QC, D])
            V.tensor_tensor(out=h2[:], in0=h2_ps[:], in1=bpe2[:], op=Alu.add)
            h2n = layernorm(h2, gpe2, bepe2, "ln2")
            h2r = ttile("h2r", [QC, D])
            S.activation(h2r[:], h2n[:], Act.Relu)

            # ---------------- phase C: attention weights ----------------
            qsT = ttile("qsT", [128, 2, QC])
            V.tensor_tensor(out=qsT[:], in0=qT[:], in1=qpT[:], op=Alu.add)
            aw_ps = ps.tile([QC, 24], F32, name="aw_ps", tag="aw_ps")
            for c in range(2):
                T.matmul(out=aw_ps[:], lhsT=qsT[:, c, :], rhs=wattn[:, c, :],
                         start=(c == 0), stop=(c == 1))
            awl = ttile("awl", [QC, 24])
            V.tensor_tensor(out=awl[:], in0=aw_ps[:], in1=battn[:], op=Alu.add)
            aw = ttile("aw", [QC, 24])
            S.activation(aw[:], awl[:], Act.Sigmoid)

            awo_ps = ps.tile([128, 4, 24], F32, name="awo_ps", tag="awo_ps")
            for j in range(4):
                T.matmul(out=awo_ps[:, j, :], lhsT=rmat[:, j, :], rhs=aw[:],
                         start=True, stop=True)
            awo = ttile("awo", [128, 4, 24])
            V.tensor_copy(awo[:], awo_ps[:])

            # ---------------- phase D: coordinate pipeline (p-ordinal) -------
            pr = PC_RANGE
            rw = ttile("rw", [128, 4, 3])
            for k in range(3):
                V.tensor_scalar(out=rw[:, :, k:k + 1], in0=rpo[:, :, k:k + 1],
                                scalar1=float(pr[3 + k] - pr[k]),
                                scalar2=float(pr[k]), op0=Alu.mult, op1=Alu.add)

            cam3 = ttile("cam3", [128, 4, 3, 6])
            tmp3 = ttile("tmp3", [128, 4, 3, 6])
            sh = [128, 4, 3, 6]
            V.tensor_tensor(out=cam3[:], in0=bc(rw[:, :, 0:1].unsqueeze(3), sh),
                            in1=bc(l2i[:, 0:1, :, :], sh), op=Alu.mult)
            V.tensor_tensor(out=tmp3[:], in0=bc(rw[:, :, 1:2].unsqueeze(3), sh),
                            in1=bc(l2i[:, 1:2, :, :], sh), op=Alu.mult)
            V.tensor_tensor(out=cam3[:], in0=cam3[:], in1=tmp3[:], op=Alu.add)
            V.tensor_tensor(out=tmp3[:], in0=bc(rw[:, :, 2:3].unsqueeze(3), sh),
                            in1=bc(l2i[:, 2:3, :, :], sh), op=Alu.mult)
            V.tensor_tensor(out=cam3[:], in0=cam3[:], in1=tmp3[:], op=Alu.add)
            V.tensor_tensor(out=cam3[:], in0=cam3[:], in1=bc(l2i[:, 3:4, :, :], sh),
                            op=Alu.add)

            zc = ttile("zc", [128, 4, 6])
            V.tensor_scalar(out=zc[:], in0=cam3[:, :, 2, :], scalar1=EPS,
                            scalar2=None, op0=Alu.max)
            rz = ttile("rz", [128, 4, 6])
            V.reciprocal(rz[:], zc[:])
            xr = ttile("xr", [128, 4, 6])
            V.tensor_tensor(out=xr[:], in0=cam3[:, :, 0, :], in1=rz[:], op=Alu.mult)
            yr = ttile("yr", [128, 4, 6])
            V.tensor_tensor(out=yr[:], in0=cam3[:, :, 1, :], in1=rz[:], op=Alu.mult)

            # mask = (z > eps) & (0 < xr < W) & (0 < yr < H)
            msk = ttile("msk", [128, 4, 6])
            mt = ttile("mt", [128, 4, 6])
            V.tensor_scalar(out=msk[:], in0=cam3[:, :, 2, :], scalar1=EPS,
                            scalar2=None, op0=Alu.is_gt)
            V.tensor_scalar(out=mt[:], in0=xr[:], scalar1=0.0, scalar2=None,
                            op0=Alu.is_gt)
            V.tensor_tensor(out=msk[:], in0=msk[:], in1=mt[:], op=Alu.mult)
            V.tensor_scalar(out=mt[:], in0=xr[:], scalar1=float(IMG_W),
                            scalar2=None, op0=Alu.is_lt)
            V.tensor_tensor(out=msk[:], in0=msk[:], in1=mt[:], op=Alu.mult)
            V.tensor_scalar(out=mt[:], in0=yr[:], scalar1=0.0, scalar2=None,
                            op0=Alu.is_gt)
            V.tensor_tensor(out=msk[:], in0=msk[:], in1=mt[:], op=Alu.mult)
            V.tensor_scalar(out=mt[:], in0=yr[:], scalar1=float(IMG_H),
                            scalar2=None, op0=Alu.is_lt)
            V.tensor_tensor(out=msk[:], in0=msk[:], in1=mt[:], op=Alu.mult)

            # maw[p, j, lev, n] = mask * aw (aw cols are n*4+l)
            sh4 = [128, 4, 4, 6]
            maw = ttile("maw", sh4)
            aw_r = awo[:].rearrange("p j (n l) -> p j l n", n=6, l=4)
            V.tensor_tensor(out=maw[:], in0=bc(msk[:].unsqueeze(2), sh4),
                            in1=aw_r, op=Alu.mult)

            # level-batched tap computation; lvlc cols:
            # 0 cWs, 1 cHs, 2 cWp1, 3 cHp1, 4 cWm1, 5 cHm1, 6 hix, 7 hiy, 8 wmul
            def lc(k):
                return bc(lvlc[:, :, k:k + 1].transpose([0, 2, 1]).unsqueeze(3), sh4)

            def flat(t):
                return t[:].rearrange("p a b c -> p (a b c)")

            def tapc_bf(k):
                return tapc[:, k:k + 1].to_broadcast([128, 96])

            def tapc_b(k):
                return bc(tapc[:, k:k + 1].unsqueeze(2).unsqueeze(3), sh4)

            xr_b = bc(xr[:].unsqueeze(2), sh4)
            yr_b = bc(yr[:].unsqueeze(2), sh4)

            pxc = ttile("pxc", sh4)
            pyc = ttile("pyc", sh4)
            V.tensor_tensor(out=pxc[:], in0=xr_b, in1=lc(0), op=Alu.mult)
            V.tensor_scalar(out=flat(pxc), in0=flat(pxc), scalar1=-0.5,
                            scalar2=-2.0, op0=Alu.add, op1=Alu.max)
            V.tensor_tensor(out=pxc[:], in0=pxc[:], in1=lc(2), op=Alu.min)
            V.tensor_tensor(out=pyc[:], in0=yr_b, in1=lc(1), op=Alu.mult)
            V.tensor_scalar(out=flat(pyc), in0=flat(pyc), scalar1=-0.5,
                            scalar2=-2.0, op0=Alu.add, op1=Alu.max)
            V.tensor_tensor(out=pyc[:], in0=pyc[:], in1=lc(3), op=Alu.min)

            # floor via int cast (robust to either rounding mode)
            def floor_frac(pc, name):
                ii = sb.tile(sh4, dt.int32, name=f"{name}_i", tag=f"{name}_i")
                V.tensor_copy(flat(ii), flat(pc))
                ff = ttile(f"{name}_f", sh4)
                V.tensor_copy(flat(ff), flat(ii))
                dg = ttile(f"{name}_d", sh4)
                V.tensor_tensor(out=flat(dg), in0=flat(ff), in1=flat(pc), op=Alu.is_gt)
                f0 = ttile(f"{name}_0", sh4)
                V.tensor_tensor(out=flat(f0), in0=flat(ff), in1=flat(dg), op=Alu.subtract)
                wf = ttile(f"{name}_w", sh4)
                V.tensor_tensor(out=flat(wf), in0=flat(pc), in1=flat(f0), op=Alu.subtract)
                return f0, wf

            x0, wx = floor_frac(pxc, "fx")
            y0, wy = floor_frac(pyc, "fy")

            xt = ttile("xt", sh4)
            V.tensor_tensor(out=flat(xt), in0=flat(x0), in1=tapc_bf(0), op=Alu.add)
            yt = ttile("yt", sh4)
            V.tensor_tensor(out=flat(yt), in0=flat(y0), in1=tapc_bf(1), op=Alu.add)

            wxt = ttile("wxt", sh4)
            V.scalar_tensor_tensor(out=flat(wxt), in0=flat(wx), scalar=tapc[:, 3:4],
                                   in1=tapc_bf(2), op0=Alu.mult, op1=Alu.add)
            wyt = ttile("wyt", sh4)
            V.scalar_tensor_tensor(out=flat(wyt), in0=flat(wy), scalar=tapc[:, 5:6],
                                   in1=tapc_bf(4), op0=Alu.mult, op1=Alu.add)

            cxl = ttile("cxl", sh4)
            V.tensor_scalar(out=flat(cxl), in0=flat(pxc), scalar1=tapc[:, 6:7],
                            scalar2=None, op0=Alu.is_ge)
            cxh = ttile("cxh", sh4)
            V.tensor_tensor(out=cxh[:], in0=pxc[:], in1=lc(6), op=Alu.is_lt)
            cyl = ttile("cyl", sh4)
            V.tensor_scalar(out=flat(cyl), in0=flat(pyc), scalar1=tapc[:, 7:8],
                            scalar2=None, op0=Alu.is_ge)
            cyh = ttile("cyh", sh4)
            V.tensor_tensor(out=cyh[:], in0=pyc[:], in1=lc(7), op=Alu.is_lt)

            wfin = ttile("wfin", sh4)
            V.tensor_tensor(out=flat(cxl), in0=flat(cxl), in1=flat(cxh), op=Alu.mult)
            V.tensor_tensor(out=flat(cyl), in0=flat(cyl), in1=flat(cyh), op=Alu.mult)
            V.tensor_tensor(out=flat(cxl), in0=flat(cxl), in1=flat(cyl), op=Alu.mult)
            V.tensor_tensor(out=flat(wfin), in0=flat(wxt), in1=flat(wyt), op=Alu.mult)
            V.tensor_tensor(out=flat(wfin), in0=flat(wfin), in1=flat(cxl), op=Alu.mult)
            V.tensor_tensor(out=flat(wfin), in0=flat(wfin), in1=flat(maw), op=Alu.mult)

            xcl = ttile("xcl", sh4)
            V.tensor_scalar(out=flat(xcl), in0=flat(xt), scalar1=0.0, scalar2=None,
                            op0=Alu.max)
            V.tensor_tensor(out=xcl[:], in0=xcl[:], in1=lc(4), op=Alu.min)
            ycl = ttile("ycl", sh4)
            V.tensor_scalar(out=flat(ycl), in0=flat(yt), scalar1=0.0, scalar2=None,
                            op0=Alu.max)
            V.tensor_tensor(out=ycl[:], in0=ycl[:], in1=lc(5), op=Alu.min)

            idxf = ttile("idxf", sh4)
            V.tensor_tensor(out=idxf[:], in0=ycl[:], in1=lc(8), op=Alu.mult)
            V.tensor_tensor(out=flat(idxf), in0=flat(idxf), in1=flat(xcl), op=Alu.add)
            base_b = baset[:].rearrange("p (l n) -> p l n", l=4, n=6).unsqueeze(1)
            V.tensor_tensor(out=idxf[:], in0=idxf[:], in1=bc(base_b, sh4),
                            op=Alu.add)

            # ---------------- phase E: wrap indices on-chip ------------------
            # gather slot i = 512m + 128j + p needs wrap[i%16, i//16], i.e.
            # wrap[p%16, 32m + 8j + p//16] = idxf[p, (j, m)]. Move partitions
            # with 8 selection matmuls (p = 16k + r -> partition r), cast into
            # a [16, 768] int16 tile with strided copies, then replicate to
            # all 128 partitions with 8 contiguous SBUF->SBUF DMAs.
            wrap16 = sb.tile([16, 24, 4, 8], dt.int16, name="wrap16",
                             tag="wrap16")
            idxf_f = idxf[:].rearrange("p a b c -> p (a b c)")  # [128, 96]
            for k in range(8):
                pw = pstr.tile([16, 96], F32, name=f"pw{k}", tag="pwrap")
                T.matmul(out=pw[:], lhsT=smat[:, k, :], rhs=idxf_f,
                         start=True, stop=True)
                V.tensor_copy(
                    wrap16[:, :, :, k].transpose([0, 2, 1]),
                    pw[:].rearrange("r (j m) -> r j m", j=4, m=24))
            wrap = sb.tile([128, NSLOT // 16], dt.int16, name="wrap", tag="wrap")
            w16f = wrap16[:].rearrange("r a b c -> r (a b c)")
            for k in range(8):
                nc.sync.dma_start(wrap[16 * k:16 * (k + 1), :], w16f)

            # ---------------- phase F: gather + weight + reduce --------------
            G.load_library(library_config.mlp)
            fused_ps = ps.tile([QC, D], F32, name="fused_ps", tag="fused_ps")
            n_mm = 96
            mm_i = 0
            for gi, (lev, c0, c1) in enumerate(GROUPS):
                nm = c1 - c0
                nidx = nm * 512
                m0 = lev * 6 + c0
                gout = gpool.tile([128, 4 * nm, D], dt.bfloat16, name=f"g{gi}",
                                  tag="gout", padded_shape=[128, 24, D])
                if os.environ.get("K_NOGATHER") == "1":
                    V.memset(gout[:], 0.25)
                else:
                    G.dma_gather(
                        out_ap=gout[:],
                        in_ap=tabs[gi][:],
                        idxs_ap=wrap[:, m0 * 32:(m0 + nm) * 32],
                        num_idxs=nidx,
                        num_idxs_reg=nidx,
                        elem_size=D,
                        single_packet=False,
                    )
                if dbg and gi == 0:
                    nc.sync.dma_start(dbg_g6[:], gout[:, 0:4, :])
                for mloc in range(nm):
                    n_cam = c0 + mloc
                    wg = wgpool.tile([128, 4, D], red_dt, name=f"wg{gi}_{mloc}",
                                     tag="wg")
                    for j in range(4):
                        V.tensor_scalar(out=wg[:, j, :],
                                        in0=gout[:, 4 * mloc + j, :],
                                        scalar1=wfin[:, j, lev, n_cam:n_cam + 1],
                                        scalar2=None, op0=Alu.mult)
                    for j in range(4):
                        T.matmul(out=fused_ps[:],
                                 lhsT=emat[:, j, :], rhs=wg[:, j, :],
                                 start=(mm_i == 0), stop=(mm_i == n_mm - 1))
                        mm_i += 1

            # ---------------- phase G: output projection ---------------------
            fused = ttile("fused", [QC, D])
            V.tensor_copy(fused[:], fused_ps[:])
            fusedT = ttile("fusedT", [128, 2, QC])
            for c in range(2):
                trp2 = pstr.tile([128, 128], F32, name="trp2", tag="trp")
                T.transpose(out=trp2[:], in_=fused[:, c * 128:(c + 1) * 128],
                            identity=ident[:])
                V.tensor_copy(fusedT[:, c, :], trp2[:])
            out_ps = ps.tile([QC, D], F32, name="out_ps", tag="out_ps")
            for c in range(2):
                T.matmul(out=out_ps[:], lhsT=fusedT[:, c, :], rhs=wout[:, c, :],
                         start=(c == 0), stop=(c == 1))
            if dbg:
                nc.sync.dma_start(dbg_aw[:], aw[:])
                nc.sync.dma_start(dbg_awo[:], awo[:])
                nc.sync.dma_start(dbg_wfin[:], wfin[:])
                nc.sync.dma_start(dbg_idx[:], idxf[:])
                nc.sync.dma_start(dbg_wrap[:], wrap[0:16, :])
                nc.sync.dma_start(dbg_fused[:], fused[:])
                nc.sync.dma_start(dbg_h2r[:], h2r[:])
            o1 = ttile("o1", [QC, D])
            V.tensor_tensor(out=o1[:], in0=out_ps[:], in1=bout[:], op=Alu.add)
            V.tensor_tensor(out=o1[:], in0=o1[:], in1=h2r[:], op=Alu.add)
            nc.sync.dma_start(out_d[:], o1[:])

    nc.compile()
    return nc


_NC_CACHE = None


def _get_program():
    global _NC_CACHE
    if _NC_CACHE is None:
        _NC_CACHE = _build_program()
    return _NC_CACHE


def _host_prep(inputs):
    """Build the shared (core-independent) and per-core input maps."""
    f32 = np.float32
    query = np.asarray(inputs["query"], f32)[0]
    query_pos = np.asarray(inputs["query_pos"], f32)[0]
    rp = np.asarray(inputs["reference_points"], f32)[0]
    l2i = np.asarray(inputs["lidar2img"], f32)[0]
    feats = [np.asarray(inputs[f"feat{i}"], f32)[0] for i in range(4)]

    def padq(x, fill):
        out = np.full((QPAD,) + x.shape[1:], fill, f32)
        out[:Q] = x
        return out

    query_p = padq(query, 0.0)
    qpos_p = padq(query_pos, 0.0)
    rp_p = padq(rp, 0.5)

    import ml_dtypes
    shared = {}
    for gi, (lev, c0, c1) in enumerate(GROUPS):
        t = feats[lev][c0:c1]
        ncam, C, H, W = t.shape
        shared[f"tab{gi}"] = np.ascontiguousarray(
            t.transpose(0, 2, 3, 1).reshape(ncam * H * W, C)).astype(
                ml_dtypes.bfloat16)

    # static matrices / constants
    E = _host_E()
    shared["emat"] = np.ascontiguousarray(E.transpose(1, 0, 2))  # [p, j, q]
    shared["rmat"] = np.ascontiguousarray(E.transpose(2, 0, 1))  # [q, j, p]

    pvec = np.arange(128)
    dx = (pvec % 4 % 2).astype(f32)
    dy = (pvec % 4 // 2).astype(f32)
    tapc = np.stack([dx, dy, 1 - dx, 2 * dx - 1, 1 - dy, 2 * dy - 1,
                     np.where(dx > 0.5, -1.0, 0.0),
                     np.where(dy > 0.5, -1.0, 0.0)], axis=1).astype(f32)
    shared["tapc"] = tapc

    lvlc = np.zeros((128, 4, 9), f32)
    for lv, (H, W) in enumerate(LVL_HW):
        lvlc[:, lv, 0] = W / IMG_W
        lvlc[:, lv, 1] = H / IMG_H
        lvlc[:, lv, 2] = W + 1.0
        lvlc[:, lv, 3] = H + 1.0
        lvlc[:, lv, 4] = W - 1.0
        lvlc[:, lv, 5] = H - 1.0
        lvlc[:, lv, 6] = np.where(dx > 0.5, W - 1.0, float(W))
        lvlc[:, lv, 7] = np.where(dy > 0.5, H - 1.0, float(H))
        lvlc[:, lv, 8] = float(W)
    shared["lvlc"] = lvlc

    base = np.zeros((128, 24), f32)
    for lv in range(4):
        H, W = LVL_HW[lv]
        for n in range(6):
            gi, mloc = _group_of(lv, n)
            base[:, lv * 6 + n] = mloc * H * W
    shared["base"] = base

    smat = np.zeros((128, 8, 16), f32)
    for p in range(128):
        smat[p, p // 16, p % 16] = 1.0
    shared["smat"] = smat

    shared["wattn"] = np.ascontiguousarray(
        np.asarray(inputs["W_attn"], f32).reshape(2, 128, 24).transpose(1, 0, 2))
    shared["battn"] = np.broadcast_to(
        np.asarray(inputs["b_attn"], f32), (QC, 24)).copy()
    shared["wout"] = np.ascontiguousarray(
        np.asarray(inputs["W_out"], f32).reshape(2, 128, D).transpose(1, 0, 2))
    shared["bout"] = np.broadcast_to(
        np.asarray(inputs["b_out"], f32), (QC, D)).copy()
    shared["wpe1"] = np.asarray(inputs["W_pe1"], f32)
    shared["wpe2"] = np.ascontiguousarray(
        np.asarray(inputs["W_pe2"], f32).reshape(2, 128, D).transpose(1, 0, 2))
    for nm, key in [("bpe1", "b_pe1"), ("gpe1", "g_pe1"), ("bepe1", "be_pe1"),
                    ("bpe2", "b_pe2"), ("gpe2", "g_pe2"), ("bepe2", "be_pe2")]:
        shared[nm] = np.broadcast_to(
            np.asarray(inputs[key], f32), (QC, D)).copy()

    # l2i replicated: [p, k, ax, n] = l2i[n, ax, k]
    l2i_r = np.broadcast_to(
        l2i.transpose(2, 1, 0)[:, :3, :][None], (QC, 4, 3, 6)).copy()
    shared["l2i"] = np.ascontiguousarray(l2i_r)

    in_maps = []
    ordmap = (32 * (pvec[:, None] * 0 + np.arange(4)[None, :]) +
              pvec[:, None] // 4)  # [128, 4] local ordinal -> local q
    for cid in range(NCORES):
        q0 = cid * QC
        m = dict(shared)
        m["qT"] = np.ascontiguousarray(
            query_p[q0:q0 + QC].T.reshape(2, 128, QC).transpose(1, 0, 2))
        m["qpT"] = np.ascontiguousarray(
            qpos_p[q0:q0 + QC].T.reshape(2, 128, QC).transpose(1, 0, 2))
        m["rpT"] = np.ascontiguousarray(rp_p[q0:q0 + QC].T)
        m["rpo"] = np.ascontiguousarray(rp_p[q0 + ordmap])
        in_maps.append(m)
    return in_maps


def kernel(**inputs):
    nc = _get_program()
    in_maps = _host_prep(inputs)
    res = run_bass_kernel_spmd(nc, in_maps, core_ids=list(range(NCORES)))
    outs = [res.results[cid]["out"] for cid in range(NCORES)]
    full = np.concatenate(outs, axis=0)[:Q]
    return full[None].astype(np.float32)


def kernel_traced(**inputs):
    """test.py helper: also returns exec_time_ns from the NTFF profile."""
    nc = _get_program()
    in_maps = _host_prep(inputs)
    res = run_bass_kernel_spmd(nc, in_maps, core_ids=list(range(NCORES)),
                               trace=True)
    outs = [res.results[cid]["out"] for cid in range(NCORES)]
    full = np.concatenate(outs, axis=0)[:Q]
    return full[None].astype(np.float32), res

